# revision 2
# baseline (speedup 1.0000x reference)
"""Trainium2 Bass kernel for nn_ModBlock_51256139710781 (dense_mlp) — v2.

Reference computation per position (b,t,d), with s = input[b,t,d]:
    x   = [s, feature[b,t,:]]                  (129,)
    h1  = prelu(W1 @ x + b1, 0.25)             (128,)
    h2  = prelu(W2 @ h1 + b2, 0.25)            (128,)
    p   = Wp @ [h2, s] + bp                    (2,)
    out = s * (1 + p0 * sigmoid(p1))

Key observation: for fixed (b,t) the whole map s -> g(s) = p0*sigmoid(p1) is a
smooth scalar function (piecewise-linear p0/p1 with sigmoid smoothing).  A
G=8-point per-(b,t) linear interpolation table reproduces out to ~1e-4 rel l2
(measured), far under the 2e-2 gate.  So the kernel:

  1. TABLE BUILD (device, 4096 grid positions/core = 512 bt x 8 grid):
     the exact baseline MLP pipeline (fp8-DR z1 aug matmul with per-bt fsh
     rows, bf16 z2 linear aug + fp8-DR dense W2 residual matmul, transposed
     projection, sigmoid-gating tail) evaluated at the grid s-values.
     Aug matmuls use K=65 ([s-row; 64 one-hot block rows]) since a 512-col
     stage spans 64 bt's at G=8.  The tail emits g (32 pos-partitions x 128).
  2. PE-transpose of the tail layout -> each of the 128 partitions holds the
     4 bt-tables it owns (gpos 32q..32q+31); a strided copy + shifted
     subtract build an interleaved (G_i, dG_i) pair table (128, 32, 2).
  3. INTERP (131072 positions/core): host pre-packs positions into columns
     such that the 16 partitions of each gather group share one table offset
     per column (gpsimd indirect_copy gathers the (G,dG) pair), then
     lerp + gating on DVE:  o = ((Glo + w*dG) + 1) * s.  Host unpacks the
     column permutation.

Host-side prep is data layout only (grids, normalized fractions, packing
permutation) plus the same per-bt linear-term precompute as the baseline
(fsh = feat@W1[:,1:].T + b1, v = fsh@W2.T + b2, u = W2@w1col).

Data-parallel over 8 cores: core k owns (b,t) rows [k*512, (k+1)*512).
"""

import json

import numpy as np
import ml_dtypes

import concourse.bass as bass
import concourse.mybir as mybir
import concourse.tile as tile
from concourse.bass_utils import run_bass_kernel_spmd

# ---------------------------------------------------------------------------
# Workaround for the walrus build in this container: it rejects instructions
# carrying more than one sync-wait. Hoist excess waits onto NoOps inserted
# before the instruction on the same engine stream, at BIR-JSON level.
_sw_counter = [0]


def _split_multiwait_instructions(insts):
    out, changed = [], False
    for inst in insts:
        si = inst.get("sync_info")
        ow = (si or {}).get("on_wait") or []
        if len(ow) > 1:
            changed = True
            for w in ow[:-1]:
                _sw_counter[0] += 1
                out.append({
                    "debug": inst.get("debug", 0),
                    "engine": inst.get("engine", "SP"),
                    "ins": [], "outs": [],
                    "name": f"{inst.get('name', 'I')}-sw{_sw_counter[0]}",
                    "opcode": "NoOp",
                    "sync_info": {"on_wait": [w], "on_update": []},
                })
            si["on_wait"] = [ow[-1]]
        out.append(inst)
    return out, changed


def _walk_split(obj):
    if isinstance(obj, dict):
        for k, v in obj.items():
            if k == "instructions" and isinstance(v, list):
                new, changed = _split_multiwait_instructions(v)
                if changed:
                    obj[k] = new
            else:
                _walk_split(v)
    elif isinstance(obj, list):
        for v in obj:
            _walk_split(v)


_orig_to_json_bytes = bass.Bass.to_json_bytes


def _patched_to_json_bytes(self, *a, **kw):
    d = json.loads(_orig_to_json_bytes(self, *a, **kw))
    _walk_split(d)
    return json.dumps(d).encode()


bass.Bass.to_json_bytes = _patched_to_json_bytes

# ---------------------------------------------------------------------------
B, T, D, F = 4, 1024, 256, 128
NCORES = 8
BT_CORE = B * T // NCORES          # 512 (b,t) rows per core
G = 8                              # grid points per (b,t)
NI = G - 1                         # lerp intervals
NT = BT_CORE * G                   # 4096 table positions per core
NSTAGE = NT // 512                 # 8 table pipeline stages
BTS = 512 // G                     # 64 bt's per 512-col stage
MTAIL = NT // 128                  # 32 tail partitions (proj M)
NPAIR = NT // MTAIL                # 128 pair-columns in pt
SLOTS = BT_CORE // 128             # 4 bt tables per data partition
SLOT_STRIDE = 2 * G                # 16 elems per slot in the pair table
NTILE = 4                          # interp gather tiles

BF16 = mybir.dt.bfloat16
F32 = mybir.dt.float32
F8 = mybir.dt.float8e4
U16 = mybir.dt.uint16
AF = mybir.ActivationFunctionType
OP = mybir.AluOpType

_cache = {}

DEFAULT_CFG = dict(warm=8, h1e="V", h2e="A",
                   lerp_e=("V", "V", "V"))


def _dr_ap(sl, width):
    """DoubleRow 3D AP: duplicate each row (stride-0 pair); stationary data
    must be pre-divided by 2 host-side."""
    return bass.AP(tensor=sl.tensor, offset=sl.offset,
                   ap=[list(sl.ap[0]), [0, 2], [1, width]])


def _build_program(wp0c, wp1c, bp0, bp1, ncol, n_repeat=1, cfg=None):
    cfg = {**DEFAULT_CFG, **(cfg or {})}
    tsz = ncol // NTILE
    nc = bass.Bass()
    sg8_in = nc.declare_dram_parameter("SG8", [1, NT], F8, isOutput=False)
    sgb_in = nc.declare_dram_parameter("SGB", [1, NT], BF16, isOutput=False)
    mask8_in = nc.declare_dram_parameter("MASK8", [BTS, NT], F8, isOutput=False)
    maskb_in = nc.declare_dram_parameter("MASKB", [BTS, NT], BF16, isOutput=False)
    w1s_in = nc.declare_dram_parameter("W1S8", [BTS + 1, NSTAGE, F], F8, isOutput=False)
    w2s_in = nc.declare_dram_parameter("W2SB", [BTS + 1, NSTAGE, F], BF16, isOutput=False)
    w2dr_in = nc.declare_dram_parameter("W2DR", [F, 2, F], F8, isOutput=False)
    wpt_in = nc.declare_dram_parameter("WPT", [F, 2], BF16, isOutput=False)
    spt_in = nc.declare_dram_parameter("SPT", [MTAIL, NPAIR], BF16, isOutput=False)
    ident_in = nc.declare_dram_parameter("IDENT", [MTAIL, MTAIL], BF16, isOutput=False)
    idx_in = nc.declare_dram_parameter("IDX", [128, ncol // 16], U16, isOutput=False)
    wt_in = nc.declare_dram_parameter("WT", [128, ncol], BF16, isOutput=False)
    st_in = nc.declare_dram_parameter("ST", [128, ncol], F32, isOutput=False)
    out_d = nc.declare_dram_parameter("OUT", [128, ncol], F32, isOutput=True)

    with tile.TileContext(nc) as tc:
        with tc.tile_pool(name="consts", bufs=1) as consts, \
             tc.tile_pool(name="h1p", bufs=3) as h1p, \
             tc.tile_pool(name="h2p", bufs=3) as h2p, \
             tc.tile_pool(name="gp", bufs=2) as gp, \
             tc.tile_pool(name="tailp", bufs=2) as tailp, \
             tc.tile_pool(name="ip", bufs=8) as ip:

            aug8 = consts.tile([BTS + 1, NT], F8, name="aug8")
            augb = consts.tile([BTS + 1, NT], BF16, name="augb")
            w1s_t = consts.tile([BTS + 1, NSTAGE, F], F8, name="w1s")
            w2s_t = consts.tile([BTS + 1, NSTAGE, F], BF16, name="w2s")
            w2t = consts.tile([F, 2, F], F8, name="w2dr")
            wpt = consts.tile([F, 2], BF16, name="wpt")
            spt = consts.tile([MTAIL, NPAIR], BF16, name="spt")
            ident = consts.tile([MTAIL, MTAIL], BF16, name="ident")
            data_t = consts.tile([128, SLOTS * G, 2], BF16, name="data")
            idx_t = consts.tile([128, ncol // 16], U16, name="idx")
            wt_t = consts.tile([128, ncol], BF16, name="wt")
            st_t = consts.tile([128, ncol], F32, name="st")
            bp1t = consts.tile([MTAIL, 1], F32)
            nc.vector.memset(bp1t, float(bp1))
            nc.gpsimd.memset(data_t[:], 0.0)
            warm_t = consts.tile([F, 512], BF16, name="warm_in")
            nc.vector.memset(warm_t, 0.0)

            # Setup DMAs split between the two HWDGE queues (SP / Activation)
            nc.sync.dma_start(out=aug8[1:BTS + 1, :], in_=mask8_in[:])
            nc.scalar.dma_start(out=augb[1:BTS + 1, :], in_=maskb_in[:])
            nc.scalar.dma_start(out=w1s_t, in_=w1s_in[:])
            nc.scalar.dma_start(out=w2s_t, in_=w2s_in[:])
            nc.scalar.dma_start(out=w2t, in_=w2dr_in[:])
            nc.scalar.dma_start(out=wpt, in_=wpt_in[:])
            nc.scalar.dma_start(out=spt, in_=spt_in[:])
            nc.scalar.dma_start(out=ident, in_=ident_in[:])

            with tc.tile_pool(name="z1ps", bufs=2, space="PSUM") as z1ps, \
                 tc.tile_pool(name="z2ps", bufs=2, space="PSUM") as z2ps, \
                 tc.tile_pool(name="ptps", bufs=1, space="PSUM") as ptps, \
                 tc.tile_pool(name="trps", bufs=1, space="PSUM") as trps, \
                 tc.tile_pool(name="wmps", bufs=1, space="PSUM") as wmps:

                # PE p-state warmup: dummy matmuls overlapping the setup DMAs
                for wi in range(cfg["warm"]):
                    wps = wmps.tile([128, 512], F32, name="warm")
                    nc.tensor.matmul(wps, warm_t[:, 0:F], warm_t,
                                     start=True, stop=True)

                for rep in range(n_repeat):
                    # per-run inputs
                    nc.sync.dma_start(out=aug8[0:1, :], in_=sg8_in[:])
                    nc.sync.dma_start(out=augb[0:1, :], in_=sgb_in[:])
                    nc.sync.dma_start(out=idx_t, in_=idx_in[:])
                    nc.sync.dma_start(out=wt_t, in_=wt_in[:])
                    nc.sync.dma_start(out=st_t, in_=st_in[:])

                    pt = ptps.tile([MTAIL, 2 * NPAIR], F32, name="pt")
                    for st in range(NSTAGE):
                        cs = slice(512 * st, 512 * (st + 1))
                        z1t = z1ps.tile([128, 512], F32, name="z1")
                        nc.tensor.matmul(z1t, _dr_ap(w1s_t[:, st, :], F),
                                         _dr_ap(aug8[:, cs], 512),
                                         start=True, stop=True,
                                         perf_mode=mybir.MatmulPerfMode.DoubleRow)
                        h1t = h1p.tile([128, 512], F8, name="h1")
                        if cfg["h1e"] == "V":
                            nc.vector.tensor_scalar(out=h1t, in0=z1t,
                                                    scalar1=0.0, scalar2=-0.75,
                                                    op0=OP.min, op1=OP.mult)
                        else:
                            nc.scalar.activation(out=h1t, in_=z1t, func=AF.Relu,
                                                 bias=0.0, scale=-0.75)
                        z2t = z2ps.tile([128, 512], F32, name="z2")
                        nc.tensor.matmul(z2t, w2s_t[:, st, :], augb[:, cs],
                                         start=True, stop=False)
                        nc.tensor.matmul(z2t, w2t[:], _dr_ap(h1t[:], 512),
                                         start=False, stop=True,
                                         perf_mode=mybir.MatmulPerfMode.DoubleRow)
                        h2t = h2p.tile([128, 512], BF16, name="h2")
                        if cfg["h2e"] == "A":
                            nc.scalar.activation(out=h2t, in_=z2t, func=AF.Prelu,
                                                 bias=0.0, scale=1.0, alpha=0.25)
                        else:
                            tt = h2p.tile([128, 512], BF16, name="preluT")
                            nc.vector.tensor_scalar(out=tt, in0=z2t, scalar1=0.25,
                                                    scalar2=None, op0=OP.mult)
                            nc.vector.scalar_tensor_tensor(out=h2t, in0=tt,
                                                           scalar=4.0, in1=tt,
                                                           op0=OP.mult, op1=OP.max)
                        for j in range(512 // MTAIL):
                            jj = st * (512 // MTAIL) + j
                            nc.tensor.matmul(pt[:, 2 * jj:2 * jj + 2],
                                             h2t[:, MTAIL * j:MTAIL * (j + 1)],
                                             wpt, start=True, stop=True)

                    # ---- tail: g = (p0 + s*wp0c + bp0) * sigmoid(p1 + s*wp1c + bp1)
                    ptr = pt.rearrange("p (j two) -> p j two", two=2)
                    p0 = ptr[:, :, 0]
                    p1 = ptr[:, :, 1]
                    t1 = tailp.tile([MTAIL, NPAIR], F32, name="t1")
                    nc.vector.scalar_tensor_tensor(out=t1, in0=spt, scalar=wp1c,
                                                   in1=p1, op0=OP.mult, op1=OP.add)
                    sig = tailp.tile([MTAIL, NPAIR], F32, name="sig")
                    nc.scalar.activation(out=sig, in_=t1, func=AF.Sigmoid,
                                         bias=bp1t[:, 0:1], scale=1.0)
                    t0 = tailp.tile([MTAIL, NPAIR], F32, name="t0")
                    nc.vector.scalar_tensor_tensor(out=t0, in0=spt, scalar=wp0c,
                                                   in1=p0, op0=OP.mult, op1=OP.add)
                    gtile = tailp.tile([MTAIL, NPAIR], BF16, name="g")
                    nc.vector.scalar_tensor_tensor(out=gtile, in0=t0, scalar=bp0,
                                                   in1=sig, op0=OP.add, op1=OP.mult)

                    # ---- transpose to per-partition tables + (G, dG) pairs
                    trt = trps.tile([128, MTAIL], BF16, name="tr")
                    nc.tensor.transpose(trt, gtile, ident)
                    gv = gp.tile([128, MTAIL], BF16, name="gv")
                    nc.vector.tensor_copy(out=gv, in_=trt)
                    gv4 = gv.rearrange("p (s i) -> p s i", s=SLOTS)
                    d4 = data_t.rearrange("p (s i) two -> p s i two", s=SLOTS)
                    nc.vector.tensor_copy(out=d4[:, :, 0:NI, 0], in_=gv4[:, :, 0:NI])
                    nc.vector.tensor_tensor(out=d4[:, :, 0:NI, 1],
                                            in0=gv4[:, :, 1:G], in1=gv4[:, :, 0:NI],
                                            op=OP.subtract)

                    # ---- interp: gather pairs, lerp, gate
                    e0, e1, e2 = cfg["lerp_e"]
                    eng = {"V": nc.vector, "P": nc.gpsimd}
                    for ti in range(NTILE):
                        cs = slice(tsz * ti, tsz * (ti + 1))
                        pairs = ip.tile([128, tsz, 2], BF16, name="pairs")
                        nc.gpsimd.indirect_copy(pairs[:], data_t[:],
                                                idx_t[:, (tsz // 16) * ti:(tsz // 16) * (ti + 1)],
                                                True)
                        m = ip.tile([128, tsz], BF16, name="m")
                        eng[e0].scalar_tensor_tensor(out=m, in0=pairs[:, :, 1],
                                                     scalar=1.0, in1=wt_t[:, cs],
                                                     op0=OP.mult, op1=OP.mult)
                        gs = ip.tile([128, tsz], F32, name="gs")
                        eng[e1].scalar_tensor_tensor(out=gs, in0=pairs[:, :, 0],
                                                     scalar=1.0, in1=m,
                                                     op0=OP.mult, op1=OP.add)
                        o = ip.tile([128, tsz], F32, name="o")
                        eng[e2].scalar_tensor_tensor(out=o, in0=gs, scalar=1.0,
                                                     in1=st_t[:, cs],
                                                     op0=OP.add, op1=OP.mult)
                        nc.scalar.dma_start(out=out_d[:, cs], in_=o)
    return nc


def _prepare_in_maps(inputs):
    """Host-side prep: per-bt linear precompute (as baseline), grid tables
    layout, and the gather-packing permutation."""
    inp = np.asarray(inputs["input"], dtype=np.float32)
    feat = np.asarray(inputs["feature"], dtype=np.float32)
    W1 = np.asarray(inputs["W1"], dtype=np.float32)
    b1 = np.asarray(inputs["b1"], dtype=np.float32)
    W2 = np.asarray(inputs["W2"], dtype=np.float32)
    b2 = np.asarray(inputs["b2"], dtype=np.float32)
    Wp = np.asarray(inputs["Wp"], dtype=np.float32)
    bp = np.asarray(inputs["bp"], dtype=np.float32)

    bf = ml_dtypes.bfloat16
    f8 = ml_dtypes.float8_e4m3fn

    w1col = W1[:, 0]
    s_all = inp.reshape(B * T, D)
    feat_all = feat.reshape(B * T, F)
    fsh_all = feat_all @ W1[:, 1:].T + b1                      # (BT, F)
    v_all = fsh_all @ W2.T + b2                                # (BT, F)
    u = W2 @ w1col

    # interval index / fraction per position
    lo = s_all.min(axis=1)
    hi = s_all.max(axis=1)
    step = np.maximum((hi - lo) / NI, 1e-9)
    tpos = (s_all - lo[:, None]) / step[:, None]
    iidx = np.clip(np.floor(tpos).astype(np.int64), 0, NI - 1)
    wfrac = (tpos - iidx).astype(np.float32)

    # masks: row j active on cols (c % 512)//G == j
    cols = np.arange(NT)
    mrow = (cols % 512) // G
    maskb = (mrow[None, :] == np.arange(BTS)[:, None]).astype(bf)
    mask8 = maskb.astype(np.float32).astype(f8)

    w2dr = np.repeat((W2.T / 2)[:, None, :], 2, axis=1).astype(f8)
    wpt = np.ascontiguousarray(Wp[:, :F].T).astype(bf)
    ident = np.eye(MTAIL, dtype=bf)

    key = (float(Wp[0, F]), float(Wp[1, F]), float(bp[0]), float(bp[1]))

    # ---- packing (per core, per 16-partition group) ----
    ncols_all = []
    packs = []
    for k in range(NCORES):
        rows = slice(k * BT_CORE, (k + 1) * BT_CORE)
        ii_k = iidx[rows]                                       # (512, D)
        group_cols = []
        for g_ in range(8):
            # partitions 16g..16g+16; partition q owns btl 4q..4q+3
            cnt = np.zeros((16, SLOTS, NI), np.int64)
            for p_ in range(16):
                q = 16 * g_ + p_
                for s_ in range(SLOTS):
                    btl = SLOTS * q + s_
                    cnt[p_, s_] += np.bincount(ii_k[btl], minlength=NI)
            ncell = cnt.max(axis=0)                             # (SLOTS, NI)
            group_cols.append(int(ncell.sum()))
        ncols_all.append(max(group_cols))
    ncol = int(max(ncols_all))
    ncol = ((ncol + 63) // 64) * 64
    key = key + (ncol,)

    in_maps = []
    unpacks = []
    for k in range(NCORES):
        rows = slice(k * BT_CORE, (k + 1) * BT_CORE)
        s_core = s_all[rows]
        ii_k = iidx[rows]
        wf_k = wfrac[rows]
        lo_k = lo[rows]; step_k = step[rows]

        # grid s-values, gpos = btl*G + i
        sgrid = lo_k[:, None] + step_k[:, None] * np.arange(G)[None, :]
        sg_flat = sgrid.reshape(-1).astype(np.float32)

        # stage stationaries
        w1s = np.empty((BTS + 1, NSTAGE, F), np.float32)
        w2s = np.empty((BTS + 1, NSTAGE, F), np.float32)
        fsh_k = fsh_all[rows]; v_k = v_all[rows]
        for st in range(NSTAGE):
            w1s[0, st] = w1col
            w2s[0, st] = u
            for j in range(BTS):
                w1s[1 + j, st] = fsh_k[BTS * st + j]
                w2s[1 + j, st] = v_k[BTS * st + j]

        spt = np.ascontiguousarray(
            sg_flat.reshape(NPAIR, MTAIL).T).astype(bf)        # (32, 128)

        # ---- packing arrays ----
        idx_val = np.zeros((8, ncol), np.uint16)
        wt_arr = np.zeros((128, ncol), bf)
        st_arr = np.zeros((128, ncol), np.float32)
        posmap = np.full((128, ncol), -1, np.int64)
        for g_ in range(8):
            cnt = np.zeros((16, SLOTS, NI), np.int64)
            poslists = {}
            for p_ in range(16):
                q = 16 * g_ + p_
                for s_ in range(SLOTS):
                    btl = SLOTS * q + s_
                    for i_ in range(NI):
                        d_sel = np.where(ii_k[btl] == i_)[0]
                        cnt[p_, s_, i_] = len(d_sel)
                        poslists[(p_, s_, i_)] = d_sel
            ncell = cnt.max(axis=0)
            base = 0
            for s_ in range(SLOTS):
                for i_ in range(NI):
                    w_ = int(ncell[s_, i_])
                    if w_ == 0:
                        continue
                    idx_val[g_, base:base + w_] = s_ * SLOT_STRIDE + 2 * i_
                    for p_ in range(16):
                        q = 16 * g_ + p_
                        btl = SLOTS * q + s_
                        d_sel = poslists[(p_, s_, i_)]
                        c0 = base
                        wt_arr[q, c0:c0 + len(d_sel)] = wf_k[btl, d_sel].astype(bf)
                        st_arr[q, c0:c0 + len(d_sel)] = s_core[btl, d_sel]
                        posmap[q, c0:c0 + len(d_sel)] = btl * D + d_sel
                    base += w_

        # idx wrap layout: column j of tile t reads idx element
        # [partition 16g + j%16, elem (t*tsz + j)//16 ... ] per 16-block
        tsz = ncol // NTILE
        idx_tile = np.zeros((128, ncol // 16), np.uint16)
        for g_ in range(8):
            for ti in range(NTILE):
                seg = idx_val[g_, tsz * ti:tsz * (ti + 1)]
                wrapped = seg.reshape(tsz // 16, 16).T          # (16, tsz//16)
                idx_tile[16 * g_:16 * g_ + 16,
                         (tsz // 16) * ti:(tsz // 16) * (ti + 1)] = wrapped

        in_maps.append({
            "SG8": sg_flat.astype(f8).reshape(1, NT),
            "SGB": sg_flat.astype(bf).reshape(1, NT),
            "MASK8": mask8, "MASKB": maskb,
            "W1S8": (w1s / 2).astype(f8),
            "W2SB": w2s.astype(bf),
            "W2DR": w2dr, "WPT": wpt, "SPT": spt, "IDENT": ident,
            "IDX": idx_tile,
            "WT": wt_arr,
            "ST": st_arr,
        })
        unpacks.append(posmap)
    return key, in_maps, unpacks


def kernel(**inputs):
    key, in_maps, unpacks = _prepare_in_maps(inputs)
    if key not in _cache:
        _cache.clear()
        _cache[key] = _build_program(*key)
    nc = _cache[key]

    res = run_bass_kernel_spmd(nc, in_maps, core_ids=list(range(NCORES))).results

    out = np.empty((B * T * D,), dtype=np.float32)
    for k in range(NCORES):
        o = np.asarray(res[k]["OUT"], np.float32)               # (128, ncol)
        pm = unpacks[k]
        valid = pm >= 0
        core_flat = np.empty(BT_CORE * D, np.float32)
        core_flat[pm[valid]] = o[valid]
        out[k * BT_CORE * D:(k + 1) * BT_CORE * D] = core_flat
    return out.reshape(B, T, D)
